# revision 32
# baseline (speedup 1.0000x reference)
"""Trainium2 Bass kernel for the per-head channel-attention module.

Math (per batch b, all fp32):
  Q = emb @ Wq[h].T, K = emb @ Wk[h].T        [N, C] each
  scores_h = Q.T @ K / sqrt(C)                [C, C]
  probs = softmax(InstanceNorm(scores), -1)
  weights = mean_h probs                      [C, C]   (output 2)
  O1 = (1/H sum_h probs_h @ V_h-chain) @ Wout [N, C]   (output 1)

Key restructure: scores contract over tokens N, so
  scores_h = Wq_h @ G' @ Wk_h.T  with  G' = (emb.T @ emb) / sqrt(C)
and the output path collapses to
  O1 = emb @ Z;  S'[i,c] = sum_h (probs_h @ Wv_h)[i,c];
  Z[c,d] = (1/H) sum_i S'[i,c] * Wout[d,i]
~11.3 GFLOP/batch -> ~1.5 GFLOP/batch.  InstanceNorm's mean subtraction
cancels inside the row softmax, so only r = rsqrt(var+eps) is needed.
G is exactly symmetric, so its lower-left block is a transpose copy.

Sharding: data-parallel, one batch per NeuronCore (B=8, 8 cores).
Host pre-transposes emb (embT) and pre-swizzles the weight matrices into
one SBUF-layout buffer so every DMA is a few large contiguous runs.
Per-core outputs: o1T = O1[b].T (host transposes back) and wts = weights[b].
"""

import os

import numpy as np

import concourse.bacc as bacc
import concourse.bass as bass
import concourse.mybir as mybir
import concourse.tile as tile
from concourse.bass import _add_dep_helper
from concourse.bass_utils import run_bass_kernel_spmd
from concourse.masks import make_identity

B, N, C, H = 8, 4096, 256, 4
EPS = 1e-5
P = 128
TC = C // P          # 2 c-tiles
KT = N // P          # 32 token-tiles
NCH = N // 512       # 8 chunks of 512 tokens for the final matmul
EMB_CHUNKS = 4
F32 = mybir.dt.float32

# Matmul dtype knobs. float32 = exact (4 cyc/row); float32r = fast (~2 cyc/row
# measured) with relaxed multiply precision (~2e-4 rel err end to end).
_DT_MAP = {"float32": mybir.dt.float32, "float32r": mybir.dt.float32r}
MM_BIG = _DT_MAP[os.environ.get("ATT_MM_BIG", "float32r")]     # Gram + O1
MM_SMALL = _DT_MAP[os.environ.get("ATT_MM_SMALL", "float32r")]  # 256^3 matmuls

# weight buffer layout (per-partition f32 element offsets)
WQ_OFF = 0
WK_OFF = WQ_OFF + H * TC * C     # 2048
WV_OFF = WK_OFF + H * TC * C     # 4096
WO_OFF = WV_OFF + H * TC * C     # 6144
WBUF_W = WO_OFF + TC * C         # 6656


def host_pack_weights(Wq, Wk, Wv, Wout):
    """Pack all weights into the exact [128, WBUF_W] SBUF image."""
    def swz(a):  # [X, 2, 128, Y] -> [128, X*2*Y]
        return np.ascontiguousarray(
            a.reshape(-1, TC, P, C).transpose(2, 0, 1, 3).reshape(P, -1)
        )

    wq = swz(Wq.transpose(0, 2, 1))   # [p, h*tc*d] = Wq[h, d, tc*128+p]
    wk = swz(Wk.transpose(0, 2, 1))
    wv = swz(Wv)                      # [p, h*tc*c] = Wv[h, tc*128+p, c]
    wo = swz(Wout.T[None])            # [p, tc*d] = Wout[d, tc*128+p]
    return np.ascontiguousarray(np.concatenate([wq, wk, wv, wo], axis=1))


def build_bass():
    nc = bacc.Bacc(None, target_bir_lowering=False)

    emb_h = nc.dram_tensor("emb", [N, C], MM_BIG, kind="ExternalInput")
    embT_h = nc.dram_tensor("embT", [C, N], MM_BIG, kind="ExternalInput")
    wbuf_h = nc.dram_tensor("wbuf", [P, WBUF_W], MM_SMALL, kind="ExternalInput")
    o1T_h = nc.dram_tensor("o1T", [C, N], F32, kind="ExternalOutput")
    wts_h = nc.dram_tensor("wts", [C, C], F32, kind="ExternalOutput")

    with tile.TileContext(nc) as tc:
        with (
            tc.tile_pool(name="singles", bufs=1) as singles,
            tc.tile_pool(name="perhead", bufs=2) as perhead,
            tc.tile_pool(name="outs", bufs=3) as outs,
            tc.tile_pool(name="psc", bufs=4, space="PSUM") as psc,
            tc.tile_pool(name="ps", bufs=2, space="PSUM") as ps,
            tc.tile_pool(name="acc", bufs=2, space="PSUM") as acc,
        ):
            # ---- resident SBUF tensors -------------------------------------
            emb_sb = singles.tile([P, KT, C], MM_BIG)    # emb[p*32+t, c]
            embT_sb = singles.tile([P, TC, N], MM_BIG)   # emb[n, t*128+p]
            wbuf_sb = singles.tile([P, WBUF_W], MM_SMALL)
            G_sb = singles.tile([P, TC, C], MM_SMALL)    # G/sqrt(C), [c', (tc,c)]
            S_sb = singles.tile([P, TC, C], MM_SMALL)    # S'/H
            Z_sb = singles.tile([P, TC, C], MM_BIG)
            probs_sb = singles.tile([P, 2 * H, C], F32)  # exp(r*s), unnormalized
            probsT_sb = singles.tile([P, H, TC, C], MM_SMALL, name="probsT")
            wacc_sb = singles.tile([P, TC, C], F32)
            stat_sb = singles.tile([P, H, 2], F32)       # mean | var+mean^2
            bnst_sb = singles.tile([P, H, TC, 6], F32)   # bn_stats scratch
            se_sb = singles.tile([P, 2 * H], F32)        # exp row sums
            rse_sb = singles.tile([P, 2 * H], F32)
            scal_sb = singles.tile([P, 12 * H], F32)     # chain scratch (per pair)
            ones_sb = singles.tile([P, P], F32)
            ones_r_sb = (
                singles.tile([P, P], MM_SMALL, name="ones_r")
                if MM_SMALL != F32
                else None
            )
            warm_sb = singles.tile([P, 64], F32, name="warm")
            ident_sb = singles.tile([P, P], F32)
            ident_r_sb = (
                singles.tile([P, P], MM_SMALL, name="ident_r")
                if MM_SMALL != F32
                else None
            )
            eps_sb = singles.tile([P, 1], F32)

            nc.vector.memset(ones_sb[:], 1.0)
            if ones_r_sb is not None:
                nc.vector.tensor_copy(out=ones_r_sb[:], in_=ones_sb[:])
            nc.vector.memset(eps_sb[:], EPS)
            make_identity(nc, ident_sb[:])
            # prewarm the ACT Exp table during the Gram phase so the first
            # real exp doesn't pay the ~1.3us table load
            nc.scalar.activation(
                out=eps_sb[:], in_=eps_sb[:],
                func=mybir.ActivationFunctionType.Exp,
            )
            nc.vector.memset(eps_sb[:], EPS)
            if ident_r_sb is not None:
                nc.vector.tensor_copy(out=ident_r_sb[:], in_=ident_sb[:])

            def wq_ap(h, t):
                return wbuf_sb[:, WQ_OFF + (h * TC + t) * C : WQ_OFF + (h * TC + t + 1) * C]

            def wk_ap(h, t):
                return wbuf_sb[:, WK_OFF + (h * TC + t) * C : WK_OFF + (h * TC + t + 1) * C]

            def wv_ap(h, t):
                return wbuf_sb[:, WV_OFF + (h * TC + t) * C : WV_OFF + (h * TC + t + 1) * C]

            def wo_ap(t):
                return wbuf_sb[:, WO_OFF + t * C : WO_OFF + (t + 1) * C]

            # ---- input DMAs (emb chunked so Gram starts early; embT later) -
            # the HWDGE queue drains strictly FIFO, so emission order below is
            # wire order: emb chunks feed the Gram, wk/wq arrive in time for
            # the first head, the rest follows, embT (O1-only) goes last.
            emb_dram = emb_h[:].rearrange("(p t) c -> p t c", p=P)
            bounds = [0, 3, 9, 16, 24, KT]
            for ci in range(len(bounds) - 1):
                nc.sync.dma_start(
                    out=emb_sb[:, bounds[ci] : bounds[ci + 1], :],
                    in_=emb_dram[:, bounds[ci] : bounds[ci + 1], :],
                )
            nc.sync.dma_start(
                out=wbuf_sb[:, WK_OFF:WV_OFF], in_=wbuf_h[:][:, WK_OFF:WV_OFF]
            )
            nc.sync.dma_start(
                out=wbuf_sb[:, WQ_OFF:WK_OFF], in_=wbuf_h[:][:, WQ_OFF:WK_OFF]
            )
            nc.sync.dma_start(
                out=wbuf_sb[:, WV_OFF:WBUF_W], in_=wbuf_h[:][:, WV_OFF:WBUF_W]
            )

            # PE warmup: dummy matmuls with no data deps keep the PE busy
            # through the initial DMA wait so HAM unthrottles before the Gram
            w_src = ones_r_sb if ones_r_sb is not None else ones_sb
            warm_ps = ps.tile([P, P], F32, tag="ps", name="warm_ps")
            NWARM = 48
            for i in range(NWARM):
                nc.tensor.matmul(
                    warm_ps[:],
                    lhsT=w_src[:],
                    rhs=w_src[:],
                    start=(i == 0),
                    stop=(i == NWARM - 1),
                )
            nc.vector.tensor_copy(out=warm_sb[:], in_=warm_ps[:, 0:64])

            # ---- Gram: G = emb.T @ emb, scaled by 1/sqrt(C) ----------------
            # token-partition per tile t is {p*32+t}; any partition of the
            # 4096 tokens is valid for the Gram contraction.  G is symmetric:
            # compute the upper 128 rows + lower-right block, transpose-copy
            # the rest.
            g0 = ps.tile([P, C], F32, tag="ps", name="g0")
            g1 = ps.tile([P, P], F32, tag="ps", name="g1")
            last_gram = None
            for k in range(KT):
                nc.tensor.matmul(
                    g0[:],
                    lhsT=emb_sb[:, k, 0:P],
                    rhs=emb_sb[:, k, :],
                    start=(k == 0),
                    stop=(k == KT - 1),
                )
                last_gram = nc.tensor.matmul(
                    g1[:],
                    lhsT=emb_sb[:, k, P:C],
                    rhs=emb_sb[:, k, P:C],
                    start=(k == 0),
                    stop=(k == KT - 1),
                )
            nc.vector.tensor_scalar_mul(G_sb[:, 0, :], g0[:], 1.0 / 16.0)
            nc.vector.tensor_scalar_mul(G_sb[:, 1, P:C], g1[:], 1.0 / 16.0)
            gt_ps = ps.tile([P, P], MM_SMALL, tag="ps", name="gt")
            ident_g = ident_r_sb[:] if ident_r_sb is not None else ident_sb[:]
            nc.tensor.transpose(gt_ps[:], G_sb[:, 0, P:C], ident_g)
            nc.vector.tensor_copy(out=G_sb[:, 1, 0:P], in_=gt_ps[:])

            # embT rides last on the FIFO DMA queue; it lands in the quiet
            # window before the O1 epilogue needs it
            nc.sync.dma_start(
                out=embT_sb[:], in_=embT_h[:].rearrange("(t p) n -> p t n", p=P)
            )

            # S' accumulator lives across the whole head loop
            s_acc = [
                acc.tile([P, C], F32, tag="acc", name=f"sacc{i}") for i in range(TC)
            ]

            inv_cc = 1.0 / float(C * C)
            sc_ps = [None] * H

            def emit_head_scores(h):
                U_sb = perhead.tile([P, TC, C], MM_SMALL, tag="u", name=f"u{h}")
                for mc in range(TC):
                    u_ps = ps.tile([P, C], F32, tag="ps")
                    for kc in range(TC):
                        nc.tensor.matmul(
                            u_ps[:],
                            lhsT=G_sb[:, kc, mc * P : (mc + 1) * P],
                            rhs=wk_ap(h, kc),
                            start=(kc == 0),
                            stop=(kc == TC - 1),
                        )
                    nc.vector.tensor_copy(out=U_sb[:, mc, :], in_=u_ps[:])

                p_ = psc.tile([P, TC, C], F32, tag="sc", name=f"sc{h}")
                for mi in range(TC):
                    for kc in range(TC):
                        nc.tensor.matmul(
                            p_[:, mi, :],
                            lhsT=wq_ap(h, kc)[:, mi * P : (mi + 1) * P],
                            rhs=U_sb[:, kc, :],
                            start=(kc == 0),
                            stop=(kc == TC - 1),
                        )
                sc_ps[h] = p_

                # per-partition mean/var in two DVE passes (no ACT tables)
                for mi in range(TC):
                    nc.vector.bn_stats(
                        out=bnst_sb[:, h, mi, :], in_=p_[:, mi, :]
                    )
                nc.vector.bn_aggr(out=stat_sb[:, h, 0:2], in_=bnst_sb[:, h, :, :])
                m2 = perhead.tile([P, 1], F32, tag="m2", name=f"m2_{h}")
                nc.vector.tensor_mul(
                    out=m2[:], in0=stat_sb[:, h, 0:1], in1=stat_sb[:, h, 0:1]
                )
                nc.vector.tensor_tensor(
                    out=stat_sb[:, h, 1:2], in0=stat_sb[:, h, 1:2], in1=m2[:],
                    op=mybir.AluOpType.add,
                )

            def emit_colsum():
                # cross-partition sums of [mean, var+mean^2], all four heads
                cs = ps.tile([P, H, 2], F32, tag="ps", name="cs")
                nc.tensor.matmul(
                    cs[:], lhsT=ones_sb[:], rhs=stat_sb[:], start=True, stop=True
                )
                return cs

            def emit_chain(cs):
                # combined var over the CxC map from per-partition stats:
                # var = E_p[var_p] + E_p[mean_p^2] - (E_p[mean_p])^2
                # then r = rsqrt(var + eps) via bit-seed + 3 Newton steps,
                # entirely on DVE so ACT keeps its Exp table loaded.
                # scal layout: mu|vt|var|y|vh|t1 (H cols each)
                mu = scal_sb[:, 0 * H : 1 * H]
                var = scal_sb[:, 1 * H : 2 * H]
                yy = scal_sb[:, 2 * H : 3 * H]
                vh = scal_sb[:, 3 * H : 4 * H]
                t1 = scal_sb[:, 4 * H : 5 * H]
                cssb = perhead.tile([P, H, 2], F32, tag="cssb", name="cssb")
                nc.vector.tensor_copy(out=cssb[:], in_=cs[:])
                nc.vector.tensor_scalar_mul(mu, cssb[:, :, 0], 1.0 / P)
                # var+eps = E_p[var_p + mean_p^2] - mu^2 + eps
                nc.vector.tensor_scalar(
                    out=var, in0=cssb[:, :, 1], scalar1=1.0 / P, scalar2=EPS,
                    op0=mybir.AluOpType.mult, op1=mybir.AluOpType.add,
                )
                nc.vector.tensor_mul(out=t1, in0=mu, in1=mu)
                nc.vector.tensor_tensor(
                    out=var, in0=var, in1=t1, op=mybir.AluOpType.subtract
                )
                nc.vector.tensor_scalar_mul(vh, var, 0.5)
                I32 = mybir.dt.int32
                nc.vector.tensor_scalar(
                    out=yy.bitcast(I32), in0=var.bitcast(I32),
                    scalar1=1, scalar2=None,
                    op0=mybir.AluOpType.logical_shift_right,
                )
                nc.vector.tensor_scalar(
                    out=yy.bitcast(I32), in0=yy.bitcast(I32),
                    scalar1=-1, scalar2=0x5F3759DF,
                    op0=mybir.AluOpType.mult, op1=mybir.AluOpType.add,
                )
                for _ in range(2):
                    nc.vector.tensor_mul(out=t1, in0=yy, in1=yy)
                    nc.vector.tensor_mul(out=t1, in0=t1, in1=vh)
                    nc.vector.tensor_scalar(
                        out=t1, in0=t1, scalar1=-1.0, scalar2=1.5,
                        op0=mybir.AluOpType.mult, op1=mybir.AluOpType.add,
                    )
                    nc.vector.tensor_mul(out=yy, in0=yy, in1=t1)

            def rr_ap(h):
                return scal_sb[:, 2 * H + h : 2 * H + h + 1]

            def emit_phase2(h):
                # scores*r is ~N(0,1) over the map: exp never overflows, so
                # skip the usual rowmax subtraction (it cancels in softmax)
                for mi in range(TC):
                    nc.scalar.activation(
                        out=probs_sb[:, TC * h + mi, :],
                        in_=sc_ps[h][:, mi, :],
                        func=mybir.ActivationFunctionType.Exp,
                        scale=rr_ap(h),
                        accum_out=se_sb[:, TC * h + mi : TC * h + mi + 1],
                    )
                nc.vector.reciprocal(
                    out=rse_sb[:, TC * h : TC * h + TC],
                    in_=se_sb[:, TC * h : TC * h + TC],
                )
                for ti in range(TC):
                    nc.vector.tensor_scalar_mul(
                        probs_sb[:, TC * h + ti, :],
                        probs_sb[:, TC * h + ti, :],
                        rse_sb[:, TC * h + ti : TC * h + ti + 1],
                    )
                    for tj in range(TC):
                        t_ps = ps.tile([P, P], F32, tag="ps")
                        nc.tensor.transpose(
                            t_ps[:],
                            probs_sb[:, TC * h + ti, tj * P : (tj + 1) * P],
                            ident_sb[:],
                        )
                        nc.vector.tensor_copy(
                            out=probsT_sb[:, h, tj, ti * P : (ti + 1) * P],
                            in_=t_ps[:],
                        )
                for mi in range(TC):
                    for kj in range(TC):
                        nc.tensor.matmul(
                            s_acc[mi][:],
                            lhsT=probsT_sb[:, h, kj, mi * P : (mi + 1) * P],
                            rhs=wv_ap(h, kj),
                            start=(h == 0 and kj == 0),
                            stop=(h == H - 1 and kj == TC - 1),
                        )

            for h in range(H):
                emit_head_scores(h)
            cs_all = emit_colsum()
            emit_chain(cs_all)
            warm2_ps = ps.tile([P, P], F32, tag="ps", name="warm2_ps")
            NW2 = 28
            for i in range(NW2):
                nc.tensor.matmul(
                    warm2_ps[:],
                    lhsT=w_src[:],
                    rhs=w_src[:],
                    start=(i == 0),
                    stop=(i == NW2 - 1),
                )
            nc.vector.tensor_copy(out=warm_sb[:], in_=warm2_ps[:, 0:64])
            for h in range(H):
                emit_phase2(h)

            # weights output: reduce over heads
            nc.vector.reduce_sum(
                out=wacc_sb[:],
                in_=probs_sb[:].rearrange("p (h m) j -> p m j h", h=H),
                axis=mybir.AxisListType.X,
            )
            nc.gpsimd.tensor_scalar_mul(wacc_sb[:], wacc_sb[:], 1.0 / H)
            nc.sync.dma_start(
                out=wts_h[:].rearrange("(t p) j -> p t j", p=P), in_=wacc_sb[:]
            )

            # ---- epilogue: Z then O1 ---------------------------------------
            for mi in range(TC):
                nc.vector.tensor_scalar_mul(S_sb[:, mi, :], s_acc[mi][:], 1.0 / H)
            for mc in range(TC):
                z_ps = ps.tile([P, C], F32, tag="ps")
                for ki in range(TC):
                    nc.tensor.matmul(
                        z_ps[:],
                        lhsT=S_sb[:, ki, mc * P : (mc + 1) * P],
                        rhs=wo_ap(ki),
                        start=(ki == 0),
                        stop=(ki == TC - 1),
                    )
                nc.vector.tensor_copy(out=Z_sb[:, mc, :], in_=z_ps[:])

            # O1.T[d, n] = sum_c Z[c, d] * embT[c, n]
            for md in range(TC):
                for nch in range(NCH):
                    idx = md * NCH + nch
                    if idx % 3 == 2:
                        o_ps = ps.tile([P, 512], F32, tag="ps")
                    else:
                        o_ps = psc.tile([P, 512], F32, tag="sc")
                    for kc in range(TC):
                        nc.tensor.matmul(
                            o_ps[:],
                            lhsT=Z_sb[:, kc, md * P : (md + 1) * P],
                            rhs=embT_sb[:, kc, nch * 512 : (nch + 1) * 512],
                            start=(kc == 0),
                            stop=(kc == TC - 1),
                        )
                    o_sb = outs.tile([P, 512], F32, tag="o1")
                    if idx % 2 == 0:
                        nc.vector.tensor_copy(out=o_sb[:], in_=o_ps[:])
                    else:
                        nc.scalar.copy(out=o_sb[:], in_=o_ps[:])
                    nc.sync.dma_start(
                        out=o1T_h[:][
                            md * P : (md + 1) * P, nch * 512 : (nch + 1) * 512
                        ],
                        in_=o_sb[:],
                    )

    nc.compile()
    return nc


_NC_CACHE = None


def host_in_maps(emb1, Wq, Wk, Wv, Wout):
    wbuf = host_pack_weights(Wq, Wk, Wv, Wout)
    in_maps = []
    for b in range(B):
        in_maps.append(
            {
                "emb": np.ascontiguousarray(emb1[b]),
                "embT": np.ascontiguousarray(emb1[b].T),
                "wbuf": wbuf,
            }
        )
    return in_maps


def kernel(emb1, Wq, Wk, Wv, Wout):
    global _NC_CACHE
    emb1 = np.ascontiguousarray(np.asarray(emb1, dtype=np.float32))
    Wq = np.asarray(Wq, dtype=np.float32)
    Wk = np.asarray(Wk, dtype=np.float32)
    Wv = np.asarray(Wv, dtype=np.float32)
    Wout = np.asarray(Wout, dtype=np.float32)

    if _NC_CACHE is None:
        _NC_CACHE = build_bass()
    nc = _NC_CACHE

    in_maps = host_in_maps(emb1, Wq, Wk, Wv, Wout)
    res = run_bass_kernel_spmd(nc, in_maps, core_ids=list(range(B)))

    O1 = np.empty((B, N, C), dtype=np.float32)
    weights = np.empty((B, C, C), dtype=np.float32)
    for b in range(B):
        O1[b] = res.results[b]["o1T"].T
        weights[b] = res.results[b]["wts"]
    return O1, weights


# revision 33
# speedup vs baseline: 1.0272x; 1.0272x over previous
"""Trainium2 Bass kernel for the per-head channel-attention module.

Math (per batch b, all fp32):
  Q = emb @ Wq[h].T, K = emb @ Wk[h].T        [N, C] each
  scores_h = Q.T @ K / sqrt(C)                [C, C]
  probs = softmax(InstanceNorm(scores), -1)
  weights = mean_h probs                      [C, C]   (output 2)
  O1 = (1/H sum_h probs_h @ V_h-chain) @ Wout [N, C]   (output 1)

Key restructure: scores contract over tokens N, so
  scores_h = Wq_h @ G' @ Wk_h.T  with  G' = (emb.T @ emb) / sqrt(C)
and the output path collapses to
  O1 = emb @ Z;  S'[i,c] = sum_h (probs_h @ Wv_h)[i,c];
  Z[c,d] = (1/H) sum_i S'[i,c] * Wout[d,i]
~11.3 GFLOP/batch -> ~1.5 GFLOP/batch.  InstanceNorm's mean subtraction
cancels inside the row softmax, so only r = rsqrt(var+eps) is needed.
G is exactly symmetric, so its lower-left block is a transpose copy.

Sharding: data-parallel, one batch per NeuronCore (B=8, 8 cores).
Host pre-transposes emb (embT) and pre-swizzles the weight matrices into
one SBUF-layout buffer so every DMA is a few large contiguous runs.
Per-core outputs: o1T = O1[b].T (host transposes back) and wts = weights[b].
"""

import os

import numpy as np

import concourse.bacc as bacc
import concourse.bass as bass
import concourse.mybir as mybir
import concourse.tile as tile
from concourse.bass import _add_dep_helper
from concourse.bass_utils import run_bass_kernel_spmd
from concourse.masks import make_identity

B, N, C, H = 8, 4096, 256, 4
EPS = 1e-5
P = 128
TC = C // P          # 2 c-tiles
KT = N // P          # 32 token-tiles
NCH = N // 512       # 8 chunks of 512 tokens for the final matmul
EMB_CHUNKS = 4
F32 = mybir.dt.float32

# Matmul dtype knobs. float32 = exact (4 cyc/row); float32r = fast (~2 cyc/row
# measured) with relaxed multiply precision (~2e-4 rel err end to end).
_DT_MAP = {"float32": mybir.dt.float32, "float32r": mybir.dt.float32r}
MM_BIG = _DT_MAP[os.environ.get("ATT_MM_BIG", "float32r")]     # Gram + O1
MM_SMALL = _DT_MAP[os.environ.get("ATT_MM_SMALL", "float32r")]  # 256^3 matmuls

# weight buffer layout (per-partition f32 element offsets)
WQ_OFF = 0
WK_OFF = WQ_OFF + H * TC * C     # 2048
WV_OFF = WK_OFF + H * TC * C     # 4096
WO_OFF = WV_OFF + H * TC * C     # 6144
WBUF_W = WO_OFF + TC * C         # 6656


def host_pack_weights(Wq, Wk, Wv, Wout):
    """Pack all weights into the exact [128, WBUF_W] SBUF image."""
    def swz(a):  # [X, 2, 128, Y] -> [128, X*2*Y]
        return np.ascontiguousarray(
            a.reshape(-1, TC, P, C).transpose(2, 0, 1, 3).reshape(P, -1)
        )

    wq = swz(Wq.transpose(0, 2, 1))   # [p, h*tc*d] = Wq[h, d, tc*128+p]
    wk = swz(Wk.transpose(0, 2, 1))
    wv = swz(Wv)                      # [p, h*tc*c] = Wv[h, tc*128+p, c]
    wo = swz(Wout.T[None])            # [p, tc*d] = Wout[d, tc*128+p]
    return np.ascontiguousarray(np.concatenate([wq, wk, wv, wo], axis=1))


def build_bass():
    nc = bacc.Bacc(None, target_bir_lowering=False)

    emb_h = nc.dram_tensor("emb", [N, C], MM_BIG, kind="ExternalInput")
    embT_h = nc.dram_tensor("embT", [C, N], MM_BIG, kind="ExternalInput")
    wbuf_h = nc.dram_tensor("wbuf", [P, WBUF_W], MM_SMALL, kind="ExternalInput")
    o1T_h = nc.dram_tensor("o1T", [C, N], F32, kind="ExternalOutput")
    wts_h = nc.dram_tensor("wts", [C, C], F32, kind="ExternalOutput")

    with tile.TileContext(nc) as tc:
        with (
            tc.tile_pool(name="singles", bufs=1) as singles,
            tc.tile_pool(name="perhead", bufs=2) as perhead,
            tc.tile_pool(name="outs", bufs=3) as outs,
            tc.tile_pool(name="psc", bufs=4, space="PSUM") as psc,
            tc.tile_pool(name="ps", bufs=2, space="PSUM") as ps,
            tc.tile_pool(name="acc", bufs=2, space="PSUM") as acc,
        ):
            # ---- resident SBUF tensors -------------------------------------
            emb_sb = singles.tile([P, KT, C], MM_BIG)    # emb[p*32+t, c]
            embT_sb = singles.tile([P, TC, N], MM_BIG)   # emb[n, t*128+p]
            wbuf_sb = singles.tile([P, WBUF_W], MM_SMALL)
            G_sb = singles.tile([P, TC, C], MM_SMALL)    # G/sqrt(C), [c', (tc,c)]
            S_sb = singles.tile([P, TC, C], MM_SMALL)    # S'/H
            Z_sb = singles.tile([P, TC, C], MM_BIG)
            probs_sb = singles.tile([P, 2 * H, C], F32)  # exp(r*s), unnormalized
            probsT_sb = singles.tile([P, H, TC, C], MM_SMALL, name="probsT")
            wacc_sb = singles.tile([P, TC, C], F32)
            stat_sb = singles.tile([P, H, 2], F32)       # mean | var+mean^2
            bnst_sb = singles.tile([P, H, TC, 6], F32)   # bn_stats scratch
            se_sb = singles.tile([P, 2 * H], F32)        # exp row sums
            rse_sb = singles.tile([P, 2 * H], F32)
            scal_sb = singles.tile([P, 12 * H], F32)     # chain scratch (per pair)
            ones_sb = singles.tile([P, P], F32)
            ones_r_sb = (
                singles.tile([P, P], MM_SMALL, name="ones_r")
                if MM_SMALL != F32
                else None
            )
            warm_sb = singles.tile([P, 64], F32, name="warm")
            ident_sb = singles.tile([P, P], F32)
            ident_r_sb = (
                singles.tile([P, P], MM_SMALL, name="ident_r")
                if MM_SMALL != F32
                else None
            )
            eps_sb = singles.tile([P, 1], F32)

            nc.vector.memset(ones_sb[:], 1.0)
            if ones_r_sb is not None:
                nc.vector.tensor_copy(out=ones_r_sb[:], in_=ones_sb[:])
            nc.vector.memset(eps_sb[:], EPS)
            make_identity(nc, ident_sb[:])
            # prewarm the ACT Exp table during the Gram phase so the first
            # real exp doesn't pay the ~1.3us table load
            nc.scalar.activation(
                out=eps_sb[:], in_=eps_sb[:],
                func=mybir.ActivationFunctionType.Exp,
            )
            nc.vector.memset(eps_sb[:], EPS)
            if ident_r_sb is not None:
                nc.vector.tensor_copy(out=ident_r_sb[:], in_=ident_sb[:])

            def wq_ap(h, t):
                return wbuf_sb[:, WQ_OFF + (h * TC + t) * C : WQ_OFF + (h * TC + t + 1) * C]

            def wk_ap(h, t):
                return wbuf_sb[:, WK_OFF + (h * TC + t) * C : WK_OFF + (h * TC + t + 1) * C]

            def wv_ap(h, t):
                return wbuf_sb[:, WV_OFF + (h * TC + t) * C : WV_OFF + (h * TC + t + 1) * C]

            def wo_ap(t):
                return wbuf_sb[:, WO_OFF + t * C : WO_OFF + (t + 1) * C]

            # ---- input DMAs (emb chunked so Gram starts early; embT later) -
            # the HWDGE queue drains strictly FIFO, so emission order below is
            # wire order: emb chunks feed the Gram, wk/wq arrive in time for
            # the first head, the rest follows, embT (O1-only) goes last.
            emb_dram = emb_h[:].rearrange("(p t) c -> p t c", p=P)
            bounds = [0, 3, 9, 16, 24, KT]
            for ci in range(len(bounds) - 1):
                nc.sync.dma_start(
                    out=emb_sb[:, bounds[ci] : bounds[ci + 1], :],
                    in_=emb_dram[:, bounds[ci] : bounds[ci + 1], :],
                )
            nc.sync.dma_start(
                out=wbuf_sb[:, WK_OFF:WV_OFF], in_=wbuf_h[:][:, WK_OFF:WV_OFF]
            )
            nc.sync.dma_start(
                out=wbuf_sb[:, WQ_OFF:WK_OFF], in_=wbuf_h[:][:, WQ_OFF:WK_OFF]
            )
            nc.sync.dma_start(
                out=wbuf_sb[:, WV_OFF:WBUF_W], in_=wbuf_h[:][:, WV_OFF:WBUF_W]
            )

            # PE warmup: dummy matmuls with no data deps keep the PE busy
            # through the initial DMA wait so HAM unthrottles before the Gram
            w_src = ones_r_sb if ones_r_sb is not None else ones_sb
            warm_ps = ps.tile([P, P], F32, tag="ps", name="warm_ps")
            NWARM = 48
            for i in range(NWARM):
                nc.tensor.matmul(
                    warm_ps[:],
                    lhsT=w_src[:],
                    rhs=w_src[:],
                    start=(i == 0),
                    stop=(i == NWARM - 1),
                )
            nc.vector.tensor_copy(out=warm_sb[:], in_=warm_ps[:, 0:64])

            # ---- Gram: G = emb.T @ emb, scaled by 1/sqrt(C) ----------------
            # token-partition per tile t is {p*32+t}; any partition of the
            # 4096 tokens is valid for the Gram contraction.  G is symmetric:
            # compute the upper 128 rows + lower-right block, transpose-copy
            # the rest.
            g0 = ps.tile([P, C], F32, tag="ps", name="g0")
            g1 = ps.tile([P, P], F32, tag="ps", name="g1")
            last_gram = None
            for k in range(KT):
                nc.tensor.matmul(
                    g0[:],
                    lhsT=emb_sb[:, k, 0:P],
                    rhs=emb_sb[:, k, :],
                    start=(k == 0),
                    stop=(k == KT - 1),
                )
                last_gram = nc.tensor.matmul(
                    g1[:],
                    lhsT=emb_sb[:, k, P:C],
                    rhs=emb_sb[:, k, P:C],
                    start=(k == 0),
                    stop=(k == KT - 1),
                )
            nc.vector.tensor_scalar_mul(G_sb[:, 0, :], g0[:], 1.0 / 16.0)
            nc.vector.tensor_scalar_mul(G_sb[:, 1, P:C], g1[:], 1.0 / 16.0)
            gt_ps = ps.tile([P, P], MM_SMALL, tag="ps", name="gt")
            ident_g = ident_r_sb[:] if ident_r_sb is not None else ident_sb[:]
            nc.tensor.transpose(gt_ps[:], G_sb[:, 0, P:C], ident_g)
            nc.vector.tensor_copy(out=G_sb[:, 1, 0:P], in_=gt_ps[:])

            # embT rides last on the FIFO DMA queue; it lands in the quiet
            # window before the O1 epilogue needs it
            nc.sync.dma_start(
                out=embT_sb[:], in_=embT_h[:].rearrange("(t p) n -> p t n", p=P)
            )

            # S' accumulator lives across the whole head loop
            s_acc = [
                acc.tile([P, C], F32, tag="acc", name=f"sacc{i}") for i in range(TC)
            ]

            inv_cc = 1.0 / float(C * C)
            sc_ps = [None] * H

            def emit_head_scores(h):
                U_sb = perhead.tile([P, TC, C], MM_SMALL, tag="u", name=f"u{h}")
                for mc in range(TC):
                    u_ps = ps.tile([P, C], F32, tag="ps")
                    for kc in range(TC):
                        nc.tensor.matmul(
                            u_ps[:],
                            lhsT=G_sb[:, kc, mc * P : (mc + 1) * P],
                            rhs=wk_ap(h, kc),
                            start=(kc == 0),
                            stop=(kc == TC - 1),
                        )
                    nc.vector.tensor_copy(out=U_sb[:, mc, :], in_=u_ps[:])

                p_ = psc.tile([P, TC, C], F32, tag="sc", name=f"sc{h}")
                for mi in range(TC):
                    for kc in range(TC):
                        nc.tensor.matmul(
                            p_[:, mi, :],
                            lhsT=wq_ap(h, kc)[:, mi * P : (mi + 1) * P],
                            rhs=U_sb[:, kc, :],
                            start=(kc == 0),
                            stop=(kc == TC - 1),
                        )
                sc_ps[h] = p_

                # per-partition mean/var in two DVE passes (no ACT tables)
                for mi in range(TC):
                    nc.vector.bn_stats(
                        out=bnst_sb[:, h, mi, :], in_=p_[:, mi, :]
                    )
                nc.vector.bn_aggr(out=stat_sb[:, h, 0:2], in_=bnst_sb[:, h, :, :])
                m2 = perhead.tile([P, 1], F32, tag="m2", name=f"m2_{h}")
                nc.vector.tensor_mul(
                    out=m2[:], in0=stat_sb[:, h, 0:1], in1=stat_sb[:, h, 0:1]
                )
                nc.vector.tensor_tensor(
                    out=stat_sb[:, h, 1:2], in0=stat_sb[:, h, 1:2], in1=m2[:],
                    op=mybir.AluOpType.add,
                )

            def emit_colsum():
                # cross-partition sums of [mean, var+mean^2], all four heads
                cs = ps.tile([P, H, 2], F32, tag="ps", name="cs")
                nc.tensor.matmul(
                    cs[:], lhsT=ones_sb[:], rhs=stat_sb[:], start=True, stop=True
                )
                return cs

            def emit_chain(cs):
                # combined var over the CxC map from per-partition stats:
                # var = E_p[var_p] + E_p[mean_p^2] - (E_p[mean_p])^2
                # then r = rsqrt(var + eps) via bit-seed + 3 Newton steps,
                # entirely on DVE so ACT keeps its Exp table loaded.
                # scal layout: mu|vt|var|y|vh|t1 (H cols each)
                mu = scal_sb[:, 0 * H : 1 * H]
                var = scal_sb[:, 1 * H : 2 * H]
                yy = scal_sb[:, 2 * H : 3 * H]
                vh = scal_sb[:, 3 * H : 4 * H]
                t1 = scal_sb[:, 4 * H : 5 * H]
                cssb = perhead.tile([P, H, 2], F32, tag="cssb", name="cssb")
                nc.vector.tensor_copy(out=cssb[:], in_=cs[:])
                nc.vector.tensor_scalar_mul(mu, cssb[:, :, 0], 1.0 / P)
                # var+eps = E_p[var_p + mean_p^2] - mu^2 + eps
                nc.vector.tensor_scalar(
                    out=var, in0=cssb[:, :, 1], scalar1=1.0 / P, scalar2=EPS,
                    op0=mybir.AluOpType.mult, op1=mybir.AluOpType.add,
                )
                nc.vector.tensor_mul(out=t1, in0=mu, in1=mu)
                nc.vector.tensor_tensor(
                    out=var, in0=var, in1=t1, op=mybir.AluOpType.subtract
                )
                nc.vector.tensor_scalar_mul(vh, var, 0.5)
                I32 = mybir.dt.int32
                nc.vector.tensor_scalar(
                    out=yy.bitcast(I32), in0=var.bitcast(I32),
                    scalar1=1, scalar2=None,
                    op0=mybir.AluOpType.logical_shift_right,
                )
                nc.vector.tensor_scalar(
                    out=yy.bitcast(I32), in0=yy.bitcast(I32),
                    scalar1=-1, scalar2=0x5F3759DF,
                    op0=mybir.AluOpType.mult, op1=mybir.AluOpType.add,
                )
                for _ in range(2):
                    nc.vector.tensor_mul(out=t1, in0=yy, in1=yy)
                    nc.vector.tensor_mul(out=t1, in0=t1, in1=vh)
                    nc.vector.tensor_scalar(
                        out=t1, in0=t1, scalar1=-1.0, scalar2=1.5,
                        op0=mybir.AluOpType.mult, op1=mybir.AluOpType.add,
                    )
                    nc.vector.tensor_mul(out=yy, in0=yy, in1=t1)

            def rr_ap(h):
                return scal_sb[:, 2 * H + h : 2 * H + h + 1]

            def emit_phase2(h):
                # scores*r is ~N(0,1) over the map: exp never overflows, so
                # skip the usual rowmax subtraction (it cancels in softmax)
                for mi in range(TC):
                    nc.scalar.activation(
                        out=probs_sb[:, TC * h + mi, :],
                        in_=sc_ps[h][:, mi, :],
                        func=mybir.ActivationFunctionType.Exp,
                        scale=rr_ap(h),
                        accum_out=se_sb[:, TC * h + mi : TC * h + mi + 1],
                    )
                nc.vector.reciprocal(
                    out=rse_sb[:, TC * h : TC * h + TC],
                    in_=se_sb[:, TC * h : TC * h + TC],
                )
                nc.vector.tensor_tensor(
                    out=probs_sb[:, TC * h : TC * h + TC, :],
                    in0=probs_sb[:, TC * h : TC * h + TC, :],
                    in1=rse_sb[:, TC * h : TC * h + TC, None].to_broadcast(
                        [P, TC, C]
                    ),
                    op=mybir.AluOpType.mult,
                )
                for ti in range(TC):
                    for tj in range(TC):
                        t_ps = ps.tile([P, P], F32, tag="ps")
                        nc.tensor.transpose(
                            t_ps[:],
                            probs_sb[:, TC * h + ti, tj * P : (tj + 1) * P],
                            ident_sb[:],
                        )
                        nc.vector.tensor_copy(
                            out=probsT_sb[:, h, tj, ti * P : (ti + 1) * P],
                            in_=t_ps[:],
                        )
                for mi in range(TC):
                    for kj in range(TC):
                        nc.tensor.matmul(
                            s_acc[mi][:],
                            lhsT=probsT_sb[:, h, kj, mi * P : (mi + 1) * P],
                            rhs=wv_ap(h, kj),
                            start=(h == 0 and kj == 0),
                            stop=(h == H - 1 and kj == TC - 1),
                        )

            for h in range(H):
                emit_head_scores(h)
            cs_all = emit_colsum()
            emit_chain(cs_all)
            for h in range(H):
                emit_phase2(h)

            # weights output: reduce over heads
            nc.vector.reduce_sum(
                out=wacc_sb[:],
                in_=probs_sb[:].rearrange("p (h m) j -> p m j h", h=H),
                axis=mybir.AxisListType.X,
            )
            nc.gpsimd.tensor_scalar_mul(wacc_sb[:], wacc_sb[:], 1.0 / H)
            nc.sync.dma_start(
                out=wts_h[:].rearrange("(t p) j -> p t j", p=P), in_=wacc_sb[:]
            )

            # ---- epilogue: Z then O1 ---------------------------------------
            for mi in range(TC):
                nc.vector.tensor_scalar_mul(S_sb[:, mi, :], s_acc[mi][:], 1.0 / H)
            for mc in range(TC):
                z_ps = ps.tile([P, C], F32, tag="ps")
                for ki in range(TC):
                    nc.tensor.matmul(
                        z_ps[:],
                        lhsT=S_sb[:, ki, mc * P : (mc + 1) * P],
                        rhs=wo_ap(ki),
                        start=(ki == 0),
                        stop=(ki == TC - 1),
                    )
                nc.vector.tensor_copy(out=Z_sb[:, mc, :], in_=z_ps[:])

            # O1.T[d, n] = sum_c Z[c, d] * embT[c, n]
            for md in range(TC):
                for nch in range(NCH):
                    idx = md * NCH + nch
                    if idx % 3 == 2:
                        o_ps = ps.tile([P, 512], F32, tag="ps")
                    else:
                        o_ps = psc.tile([P, 512], F32, tag="sc")
                    for kc in range(TC):
                        nc.tensor.matmul(
                            o_ps[:],
                            lhsT=Z_sb[:, kc, md * P : (md + 1) * P],
                            rhs=embT_sb[:, kc, nch * 512 : (nch + 1) * 512],
                            start=(kc == 0),
                            stop=(kc == TC - 1),
                        )
                    o_sb = outs.tile([P, 512], F32, tag="o1")
                    if idx % 2 == 0:
                        nc.vector.tensor_copy(out=o_sb[:], in_=o_ps[:])
                    else:
                        nc.scalar.copy(out=o_sb[:], in_=o_ps[:])
                    nc.sync.dma_start(
                        out=o1T_h[:][
                            md * P : (md + 1) * P, nch * 512 : (nch + 1) * 512
                        ],
                        in_=o_sb[:],
                    )

    nc.compile()
    return nc


_NC_CACHE = None


def host_in_maps(emb1, Wq, Wk, Wv, Wout):
    wbuf = host_pack_weights(Wq, Wk, Wv, Wout)
    in_maps = []
    for b in range(B):
        in_maps.append(
            {
                "emb": np.ascontiguousarray(emb1[b]),
                "embT": np.ascontiguousarray(emb1[b].T),
                "wbuf": wbuf,
            }
        )
    return in_maps


def kernel(emb1, Wq, Wk, Wv, Wout):
    global _NC_CACHE
    emb1 = np.ascontiguousarray(np.asarray(emb1, dtype=np.float32))
    Wq = np.asarray(Wq, dtype=np.float32)
    Wk = np.asarray(Wk, dtype=np.float32)
    Wv = np.asarray(Wv, dtype=np.float32)
    Wout = np.asarray(Wout, dtype=np.float32)

    if _NC_CACHE is None:
        _NC_CACHE = build_bass()
    nc = _NC_CACHE

    in_maps = host_in_maps(emb1, Wq, Wk, Wv, Wout)
    res = run_bass_kernel_spmd(nc, in_maps, core_ids=list(range(B)))

    O1 = np.empty((B, N, C), dtype=np.float32)
    weights = np.empty((B, C, C), dtype=np.float32)
    for b in range(B):
        O1[b] = res.results[b]["o1T"].T
        weights[b] = res.results[b]["wts"]
    return O1, weights
